# revision 32
# baseline (speedup 1.0000x reference)
"""GNN message-passing (std aggregator) on 8 TRN2 NeuronCores.

Math per target node n: count, S1 = sum x[src], S2 = sum x[src]^2;
mean = S1/count; var = S2/count - mean^2; std = sqrt(max(var,0)),
zeroed where count <= 1.

Strategy (edge-major, fp8 DoubleRow identity-matmul segment-sum):
  Host sorts nodes by in-degree and assigns each node one SBUF lane:
  rank r -> (global block g = r//128, lane p = r%128); block g -> core
  g%8, per-core block index i = g//8. Per block-index capacity cap_i =
  max in-degree across the 8 interleaved global blocks (degree-sorted,
  so padding is a few %), rounded up to even. Messages x[src] are
  shipped pre-gathered (host-side layout only) as fp8-e4m3 slabs
  [128 lanes, cap_i*64] per block: column group j holds lane-node's
  j-th incoming message.

  Device per group of blocks: DMA slab; squares computed on a 3-way
  ACT/DVE/GPSIMD column split; PE accumulates S1 and S2 per lane with
  wrapped-output DoubleRow matmuls (pair-identity stationary [128,2,128]
  fp8 contracts 256 rows, out AP [128, c, 64] stride-0 accumulates in
  PSUM, 2 rows/cycle); DVE finishing reads PSUM: tv = [S1|S2] * a
  (a = mask/count shipped doubled as bf16 plane), v = max(v - t*t, 0);
  ACT sqrt -> std bf16; DMA out node-major. No per-edge descriptors,
  no collectives; every engine does large unit-stride work.
"""

import numpy as np
import ml_dtypes

N_NODES = 100000
N_FEAT = 64
P = 128
NCORES = 8
NBLK = 784                # global blocks (784*128 = 100352 >= 100000)
NB = NBLK // NCORES       # 98 per-core blocks
NRANK = NBLK * P
NGROUP = 16               # DMA/compute groups per core
MMC = 8                   # slots per S1 matmul (512 moving cycles)
MMCP = 8                  # pairs per S2 DoubleRow matmul (512 cycles)

ACT_FRAC = 0.70           # squaring share on ScalarE
GP_FRAC = 0.30            # squaring share on GPSIMD (rest on DVE)

_CACHE = {}


def _build_program(caps, groups):
    import concourse.bass as bass
    import concourse.bacc as bacc
    import concourse.mybir as mybir
    import concourse.tile as tile

    F = N_FEAT
    FP8M = mybir.dt.float8e3      # e3m4: messages
    FP8S = mybir.dt.float8e4      # e4m3: squares (range to 448)
    BF16 = mybir.dt.bfloat16
    F32 = mybir.dt.float32
    AF = mybir.ActivationFunctionType
    AO = mybir.AluOpType
    PM = mybir.MatmulPerfMode

    offs = np.zeros(NB + 1, np.int64)
    np.cumsum(caps, out=offs[1:])
    tot = int(offs[-1])
    maxg = max(int(offs[b1] - offs[b0]) for b0, b1 in groups)
    maxgb = max(b1 - b0 for b0, b1 in groups)

    nc = bacc.Bacc()
    msgsd = nc.declare_dram_parameter("msgs", [P, tot * F], FP8M, isOutput=False)
    arepd = nc.declare_dram_parameter("arep", [P, NB * 2 * F], BF16,
                                      isOutput=False)
    id3d = nc.declare_dram_parameter("ident3", [P, P], FP8M, isOutput=False)
    identd = nc.declare_dram_parameter("identp", [P, 2 * P], FP8S,
                                       isOutput=False)
    outd = nc.declare_dram_parameter("out", [P, NB * F], BF16, isOutput=True)

    with tile.TileContext(nc) as tc:
        with (
            tc.tile_pool(name="const", bufs=1) as constp,
            tc.tile_pool(name="io", bufs=3) as iop,
            tc.tile_pool(name="sq", bufs=3) as sqp,
            tc.tile_pool(name="fin", bufs=2) as finp,
            tc.tile_pool(name="ov", bufs=2) as ovp,
            tc.tile_pool(name="ps", bufs=8, space="PSUM") as psump,
        ):
            ident3 = constp.tile([P, P], FP8M)
            nc.sync.dma_start(out=ident3[:], in_=id3d[:, :])
            identp = constp.tile([P, 2 * P], FP8S)
            nc.sync.dma_start(out=identp[:], in_=identd[:, :])
            lhsp = identp[:].rearrange("p (q m) -> p q m", q=2)
            # arep rides the scalar-engine HWDGE ring so the 3.2MB transfer
            # does not delay the first message slab on the sync ring
            arep = constp.tile([P, NB * 2 * F], BF16)
            nc.scalar.dma_start(out=arep[:], in_=arepd[:, :])

            for b0, b1 in groups:
                gcols = int(offs[b1] - offs[b0])
                gb = b1 - b0
                slab = iop.tile([P, maxg * F], FP8M, tag="slab")
                nc.sync.dma_start(
                    out=slab[:, : gcols * F],
                    in_=msgsd[:, int(offs[b0]) * F : int(offs[b1]) * F],
                )
                sqs = sqp.tile([P, maxg * F], FP8S, tag="sqs")
                n = gcols * F
                c1 = int(n * ACT_FRAC) // F * F
                c2 = c1 + int(n * GP_FRAC) // F * F
                h1 = c1 // 2 // F * F
                nc.scalar.activation(
                    out=sqs[:, :h1], in_=slab[:, :h1], func=AF.Square)
                nc.scalar.activation(
                    out=sqs[:, h1:c1], in_=slab[:, h1:c1], func=AF.Square)
                nc.gpsimd.tensor_tensor(
                    out=sqs[:, c1:c2], in0=slab[:, c1:c2],
                    in1=slab[:, c1:c2], op=AO.mult)
                if c2 < n:
                    nc.vector.tensor_tensor(
                        out=sqs[:, c2:n], in0=slab[:, c2:n],
                        in1=slab[:, c2:n], op=AO.mult)

                # one block per PSUM bank; per-block tv-mult right after its
                # stop keeps banks recycling fast (DVE has no square work)
                tv = finp.tile([P, maxgb * 2 * F], F32, tag="tv")
                boff = 0
                for bb, b in enumerate(range(b0, b1)):
                    cap = int(caps[b])
                    npair = cap // 2
                    ps = psump.tile([P, 2 * F], F32, tag="ps",
                                    name=f"ps_{b}")
                    # S1: standard matmul over e3m4 message slots
                    r3 = slab[:, boff * F : (boff + cap) * F].rearrange(
                        "p (c f) -> p c f", f=F)
                    dst1 = ps[:, 0:F].rearrange("p (o f) -> p o f", o=1)
                    nch1 = (cap + MMC - 1) // MMC
                    for k in range(nch1):
                        sz = min(MMC, cap - k * MMC)
                        nc.tensor.matmul(
                            out=dst1.to_broadcast([P, sz, F]),
                            lhsT=ident3[:],
                            rhs=r3[:, k * MMC : k * MMC + sz, :],
                            start=(k == 0), stop=False,
                        )
                    # S2: DoubleRow over e4m3 squares (pair planes)
                    s4 = sqs[:, boff * F : (boff + cap) * F].rearrange(
                        "p (c q f) -> p q c f", q=2, f=F)
                    dst2 = ps[:, F : 2 * F].rearrange("p (o f) -> p o f", o=1)
                    nch2 = (npair + MMCP - 1) // MMCP
                    for k in range(nch2):
                        sz = min(MMCP, npair - k * MMCP)
                        nc.tensor.matmul(
                            out=dst2.to_broadcast([P, sz, F]),
                            lhsT=lhsp,
                            rhs=s4[:, :, k * MMCP : k * MMCP + sz, :],
                            start=False, stop=(k == nch2 - 1),
                            perf_mode=PM.DoubleRow,
                        )
                    boff += cap
                    nc.vector.tensor_tensor(
                        out=tv[:, bb * 2 * F : (bb + 1) * 2 * F],
                        in0=ps[:, :],
                        in1=arep[:, b * 2 * F : (b + 1) * 2 * F],
                        op=AO.mult)
                tv3 = tv[:, : gb * 2 * F].rearrange("p (b h f) -> p b h f",
                                                    h=2, f=F)
                th = tv3[:, :, 0, :]
                vh = tv3[:, :, 1, :]
                nc.vector.tensor_tensor(out=th, in0=th, in1=th, op=AO.mult)
                nc.vector.tensor_tensor(out=vh, in0=vh, in1=th, op=AO.subtract)
                nc.vector.tensor_scalar(out=vh, in0=vh, scalar1=0.0,
                                        scalar2=None, op0=AO.max)
                s = ovp.tile([P, maxgb * F], BF16, tag="s")
                nc.scalar.activation(out=s[:, : gb * F]
                                     .rearrange("p (b f) -> p b f", f=F),
                                     in_=vh, func=AF.Sqrt)
                nc.sync.dma_start(out=outd[:, b0 * F : b1 * F],
                                  in_=s[:, : gb * F])
    return nc


def _host_prep(x, edge_index):
    bf16 = ml_dtypes.bfloat16
    fp8m = ml_dtypes.float8_e3m4
    fp8s = ml_dtypes.float8_e4m3fn
    src = np.asarray(edge_index[0], dtype=np.int64)
    tgt = np.asarray(edge_index[1], dtype=np.int64)
    n_edges = src.shape[0]

    counts = np.bincount(tgt, minlength=N_NODES)
    order = np.argsort(-counts, kind="stable")          # rank -> node
    deg_r = np.zeros(NRANK, np.int64)
    deg_r[:N_NODES] = counts[order]
    rank = np.empty(N_NODES, np.int64)
    rank[order] = np.arange(N_NODES)

    caps = np.maximum(deg_r[np.arange(NB) * NCORES * P], 2)   # per block idx
    caps = (caps + 1) // 2 * 2                                # even for pairs
    offs = np.zeros(NB + 1, np.int64)
    np.cumsum(caps, out=offs[1:])
    tot = int(offs[-1])

    # groups: contiguous blocks with ~equal total capacity; the first and
    # last are half-size for faster pipeline ramp and drain
    targets = [tot / NGROUP / 2, tot / NGROUP / 2] + \
        [tot / NGROUP] * (NGROUP - 3) + [tot / NGROUP / 2, tot / NGROUP / 2]
    groups = []
    b0 = 0
    acc = 0
    gi = 0
    for b in range(NB):
        acc += caps[b]
        tgt_sz = targets[min(gi, len(targets) - 1)]
        if acc >= tgt_sz and b + 1 < NB or b == NB - 1:
            groups.append((b0, b + 1))
            b0 = b + 1
            acc = 0
            gi += 1
    if b0 < NB:
        groups.append((b0, NB))

    # per-edge placement
    r_t = rank[tgt]
    eorder = np.argsort(r_t, kind="stable")
    rs = r_t[eorder]
    starts = np.zeros(NRANK, np.int64)
    np.cumsum(deg_r[:-1], out=starts[1:])
    j = np.arange(n_edges) - starts[rs]
    g = rs // P
    p = rs % P
    core = g % NCORES
    blk = g // NCORES
    col = offs[blk] + j
    srcs = src[eorder]

    xb = np.asarray(x, np.float32).astype(fp8m)

    # per-node scale a = mask/count, node-major, doubled [P, NB*2F]
    ranks_core = ((np.arange(NB)[:, None] * NCORES)[None, :, :]
                  + np.arange(NCORES)[:, None, None]) * P \
        + np.arange(P)[None, None, :]                   # [NCORES, NB, P]
    d_core = deg_r[ranks_core]                          # [NCORES, NB, P]
    a_core = np.where(d_core > 1, 1.0 / np.maximum(d_core, 1), 0.0)

    ident3 = np.eye(P, dtype=fp8m)
    identp = np.concatenate([np.eye(P), np.eye(P)], axis=1).astype(fp8s)
    in_maps = []
    for c in range(NCORES):
        m = core == c
        buf = np.zeros((P, tot, N_FEAT), fp8m)
        buf[p[m], col[m]] = xb[srcs[m]]
        arep = np.ascontiguousarray(
            np.broadcast_to(
                a_core[c].T[:, :, None, None], (P, NB, 2, N_FEAT)
            ).reshape(P, NB * 2 * N_FEAT).astype(bf16))
        in_maps.append({
            "msgs": buf.reshape(P, tot * N_FEAT),
            "arep": arep,
            "ident3": ident3,
            "identp": identp,
        })

    # output mapping: node_grid[c, i, p] = node id (or -1 pad)
    order_pad = np.full(NRANK, -1, np.int64)
    order_pad[:N_NODES] = order
    node_grid = order_pad[ranks_core]                   # [NCORES, NB, P]
    return caps, groups, in_maps, node_grid


def _run(x, edge_index, trace=False):
    from concourse.bass_utils import run_bass_kernel_spmd

    caps, groups, in_maps, node_grid = _host_prep(x, edge_index)
    key = (tuple(int(c) for c in caps), tuple(groups))
    if key not in _CACHE:
        nc_ = _build_program(caps, groups)
        nc_.finalize()
        _CACHE[key] = nc_
    nc = _CACHE[key]
    res = run_bass_kernel_spmd(
        nc, in_maps, core_ids=list(range(NCORES)), trace=trace)

    out_full = np.empty((N_NODES, N_FEAT), np.float32)
    for c in range(NCORES):
        oc = np.asarray(res.results[c]["out"]).astype(np.float32)
        oc = oc.reshape(P, NB, N_FEAT).transpose(1, 0, 2)   # [NB, P, F]
        ng = node_grid[c]                                   # [NB, P]
        valid = ng >= 0
        out_full[ng[valid]] = oc[valid]
    return out_full, res


def kernel(**inputs):
    out, _ = _run(inputs["x"], inputs["edge_index"], trace=False)
    return out


# revision 35
# speedup vs baseline: 1.0676x; 1.0676x over previous
"""GNN message-passing (std aggregator) on 8 TRN2 NeuronCores.

Math per target node n: count, S1 = sum x[src], S2 = sum x[src]^2;
mean = S1/count; var = S2/count - mean^2; std = sqrt(max(var,0)),
zeroed where count <= 1.

Strategy (edge-major, fp8 DoubleRow identity-matmul segment-sum):
  Host sorts nodes by in-degree and assigns each node one SBUF lane:
  rank r -> (global block g = r//128, lane p = r%128); block g -> core
  g%8, per-core block index i = g//8. Per block-index capacity cap_i =
  max in-degree across the 8 interleaved global blocks (degree-sorted,
  so padding is a few %), rounded up to even. Messages x[src] are
  shipped pre-gathered (host-side layout only) as fp8-e4m3 slabs
  [128 lanes, cap_i*64] per block: column group j holds lane-node's
  j-th incoming message.

  Device per group of blocks: DMA slab; squares computed on a 3-way
  ACT/DVE/GPSIMD column split; PE accumulates S1 and S2 per lane with
  wrapped-output DoubleRow matmuls (pair-identity stationary [128,2,128]
  fp8 contracts 256 rows, out AP [128, c, 64] stride-0 accumulates in
  PSUM, 2 rows/cycle); DVE finishing reads PSUM: tv = [S1|S2] * a
  (a = mask/count shipped doubled as bf16 plane), v = max(v - t*t, 0);
  ACT sqrt -> std bf16; DMA out node-major. No per-edge descriptors,
  no collectives; every engine does large unit-stride work.
"""

import numpy as np
import ml_dtypes

N_NODES = 100000
N_FEAT = 64
P = 128
NCORES = 8
NBLK = 784                # global blocks (784*128 = 100352 >= 100000)
NB = NBLK // NCORES       # 98 per-core blocks
NRANK = NBLK * P
NGROUP = 16               # DMA/compute groups per core
MMC = 8                   # slots per S1 matmul (512 moving cycles)
MMCP = 8                  # pairs per S2 DoubleRow matmul (512 cycles)

ACT_FRAC = 0.70           # squaring share on ScalarE
GP_FRAC = 0.30            # squaring share on GPSIMD (rest on DVE)

_CACHE = {}


def _build_program(caps, groups):
    import concourse.bass as bass
    import concourse.bacc as bacc
    import concourse.mybir as mybir
    import concourse.tile as tile

    F = N_FEAT
    FP8M = mybir.dt.float8e3      # e3m4: messages
    FP8S = mybir.dt.float8e4      # e4m3: squares (range to 448)
    BF16 = mybir.dt.bfloat16
    F32 = mybir.dt.float32
    AF = mybir.ActivationFunctionType
    AO = mybir.AluOpType
    PM = mybir.MatmulPerfMode

    offs = np.zeros(NB + 1, np.int64)
    np.cumsum(caps, out=offs[1:])
    tot = int(offs[-1])
    maxg = max(int(offs[b1] - offs[b0]) for b0, b1 in groups)
    maxgb = max(b1 - b0 for b0, b1 in groups)

    nc = bacc.Bacc()
    msgsd = nc.declare_dram_parameter("msgs", [P, tot * F], FP8M, isOutput=False)
    arepd = nc.declare_dram_parameter("arep", [P, NB * 2 * F], BF16,
                                      isOutput=False)
    id3d = nc.declare_dram_parameter("ident3", [P, P], FP8M, isOutput=False)
    identd = nc.declare_dram_parameter("identp", [P, 2 * P], FP8S,
                                       isOutput=False)
    outd = nc.declare_dram_parameter("out", [P, NB * F], BF16, isOutput=True)

    with tile.TileContext(nc) as tc:
        with (
            tc.tile_pool(name="const", bufs=1) as constp,
            tc.tile_pool(name="io", bufs=3) as iop,
            tc.tile_pool(name="sq", bufs=3) as sqp,
            tc.tile_pool(name="fin", bufs=2) as finp,
            tc.tile_pool(name="ov", bufs=2) as ovp,
            tc.tile_pool(name="ps", bufs=8, space="PSUM") as psump,
        ):
            ident3 = constp.tile([P, P], FP8M)
            nc.sync.dma_start(out=ident3[:], in_=id3d[:, :])
            identp = constp.tile([P, 2 * P], FP8S)
            nc.sync.dma_start(out=identp[:], in_=identd[:, :])
            lhsp = identp[:].rearrange("p (q m) -> p q m", q=2)
            arep = constp.tile([P, NB * 2 * F], BF16)
            nc.sync.dma_start(out=arep[:], in_=arepd[:, :])

            for b0, b1 in groups:
                gcols = int(offs[b1] - offs[b0])
                gb = b1 - b0
                slab = iop.tile([P, maxg * F], FP8M, tag="slab")
                nc.sync.dma_start(
                    out=slab[:, : gcols * F],
                    in_=msgsd[:, int(offs[b0]) * F : int(offs[b1]) * F],
                )
                sqs = sqp.tile([P, maxg * F], FP8S, tag="sqs")
                n = gcols * F
                c1 = int(n * ACT_FRAC) // F * F
                c2 = c1 + int(n * GP_FRAC) // F * F
                nc.scalar.activation(
                    out=sqs[:, :c1], in_=slab[:, :c1], func=AF.Square)
                nc.gpsimd.tensor_tensor(
                    out=sqs[:, c1:c2], in0=slab[:, c1:c2],
                    in1=slab[:, c1:c2], op=AO.mult)
                if c2 < n:
                    nc.vector.tensor_tensor(
                        out=sqs[:, c2:n], in0=slab[:, c2:n],
                        in1=slab[:, c2:n], op=AO.mult)

                # one block per PSUM bank; per-block tv-mult right after its
                # stop keeps banks recycling fast (DVE has no square work)
                tv = finp.tile([P, maxgb * 2 * F], F32, tag="tv")
                boff = 0
                for bb, b in enumerate(range(b0, b1)):
                    cap = int(caps[b])
                    npair = cap // 2
                    ps = psump.tile([P, 2 * F], F32, tag="ps",
                                    name=f"ps_{b}")
                    # S1: standard matmul over e3m4 message slots
                    r3 = slab[:, boff * F : (boff + cap) * F].rearrange(
                        "p (c f) -> p c f", f=F)
                    dst1 = ps[:, 0:F].rearrange("p (o f) -> p o f", o=1)
                    nch1 = (cap + MMC - 1) // MMC
                    for k in range(nch1):
                        sz = min(MMC, cap - k * MMC)
                        nc.tensor.matmul(
                            out=dst1.to_broadcast([P, sz, F]),
                            lhsT=ident3[:],
                            rhs=r3[:, k * MMC : k * MMC + sz, :],
                            start=(k == 0), stop=False,
                        )
                    # S2: DoubleRow over e4m3 squares (pair planes)
                    s4 = sqs[:, boff * F : (boff + cap) * F].rearrange(
                        "p (c q f) -> p q c f", q=2, f=F)
                    dst2 = ps[:, F : 2 * F].rearrange("p (o f) -> p o f", o=1)
                    nch2 = (npair + MMCP - 1) // MMCP
                    for k in range(nch2):
                        sz = min(MMCP, npair - k * MMCP)
                        nc.tensor.matmul(
                            out=dst2.to_broadcast([P, sz, F]),
                            lhsT=lhsp,
                            rhs=s4[:, :, k * MMCP : k * MMCP + sz, :],
                            start=False, stop=(k == nch2 - 1),
                            perf_mode=PM.DoubleRow,
                        )
                    boff += cap
                    nc.vector.tensor_tensor(
                        out=tv[:, bb * 2 * F : (bb + 1) * 2 * F],
                        in0=ps[:, :],
                        in1=arep[:, b * 2 * F : (b + 1) * 2 * F],
                        op=AO.mult)
                tv3 = tv[:, : gb * 2 * F].rearrange("p (b h f) -> p b h f",
                                                    h=2, f=F)
                th = tv3[:, :, 0, :]
                vh = tv3[:, :, 1, :]
                nc.vector.tensor_tensor(out=th, in0=th, in1=th, op=AO.mult)
                nc.vector.tensor_tensor(out=vh, in0=vh, in1=th, op=AO.subtract)
                nc.vector.tensor_scalar(out=vh, in0=vh, scalar1=0.0,
                                        scalar2=None, op0=AO.max)
                s = ovp.tile([P, maxgb * F], BF16, tag="s")
                nc.scalar.activation(out=s[:, : gb * F]
                                     .rearrange("p (b f) -> p b f", f=F),
                                     in_=vh, func=AF.Sqrt)
                nc.sync.dma_start(out=outd[:, b0 * F : b1 * F],
                                  in_=s[:, : gb * F])
    return nc


def _host_prep(x, edge_index):
    bf16 = ml_dtypes.bfloat16
    fp8m = ml_dtypes.float8_e3m4
    fp8s = ml_dtypes.float8_e4m3fn
    src = np.asarray(edge_index[0], dtype=np.int64)
    tgt = np.asarray(edge_index[1], dtype=np.int64)
    n_edges = src.shape[0]

    counts = np.bincount(tgt, minlength=N_NODES)
    order = np.argsort(-counts, kind="stable")          # rank -> node
    deg_r = np.zeros(NRANK, np.int64)
    deg_r[:N_NODES] = counts[order]
    rank = np.empty(N_NODES, np.int64)
    rank[order] = np.arange(N_NODES)

    caps = np.maximum(deg_r[np.arange(NB) * NCORES * P], 2)   # per block idx
    caps = (caps + 1) // 2 * 2                                # even for pairs
    offs = np.zeros(NB + 1, np.int64)
    np.cumsum(caps, out=offs[1:])
    tot = int(offs[-1])

    # groups: contiguous blocks with ~equal total capacity
    target = tot / NGROUP
    groups = []
    b0 = 0
    acc = 0
    for b in range(NB):
        acc += caps[b]
        if acc >= target and b + 1 < NB or b == NB - 1:
            groups.append((b0, b + 1))
            b0 = b + 1
            acc = 0
    if b0 < NB:
        groups.append((b0, NB))

    # per-edge placement
    r_t = rank[tgt]
    eorder = np.argsort(r_t, kind="stable")
    rs = r_t[eorder]
    starts = np.zeros(NRANK, np.int64)
    np.cumsum(deg_r[:-1], out=starts[1:])
    j = np.arange(n_edges) - starts[rs]
    g = rs // P
    p = rs % P
    core = g % NCORES
    blk = g // NCORES
    col = offs[blk] + j
    srcs = src[eorder]

    xb = np.asarray(x, np.float32).astype(fp8m)

    # per-node scale a = mask/count, node-major, doubled [P, NB*2F]
    ranks_core = ((np.arange(NB)[:, None] * NCORES)[None, :, :]
                  + np.arange(NCORES)[:, None, None]) * P \
        + np.arange(P)[None, None, :]                   # [NCORES, NB, P]
    d_core = deg_r[ranks_core]                          # [NCORES, NB, P]
    a_core = np.where(d_core > 1, 1.0 / np.maximum(d_core, 1), 0.0)

    ident3 = np.eye(P, dtype=fp8m)
    identp = np.concatenate([np.eye(P), np.eye(P)], axis=1).astype(fp8s)
    in_maps = []
    for c in range(NCORES):
        m = core == c
        buf = np.zeros((P, tot, N_FEAT), fp8m)
        buf[p[m], col[m]] = xb[srcs[m]]
        arep = np.ascontiguousarray(
            np.broadcast_to(
                a_core[c].T[:, :, None, None], (P, NB, 2, N_FEAT)
            ).reshape(P, NB * 2 * N_FEAT).astype(bf16))
        in_maps.append({
            "msgs": buf.reshape(P, tot * N_FEAT),
            "arep": arep,
            "ident3": ident3,
            "identp": identp,
        })

    # output mapping: node_grid[c, i, p] = node id (or -1 pad)
    order_pad = np.full(NRANK, -1, np.int64)
    order_pad[:N_NODES] = order
    node_grid = order_pad[ranks_core]                   # [NCORES, NB, P]
    return caps, groups, in_maps, node_grid


def _run(x, edge_index, trace=False):
    from concourse.bass_utils import run_bass_kernel_spmd

    caps, groups, in_maps, node_grid = _host_prep(x, edge_index)
    key = (tuple(int(c) for c in caps), tuple(groups))
    if key not in _CACHE:
        nc_ = _build_program(caps, groups)
        nc_.finalize()
        _CACHE[key] = nc_
    nc = _CACHE[key]
    res = run_bass_kernel_spmd(
        nc, in_maps, core_ids=list(range(NCORES)), trace=trace)

    out_full = np.empty((N_NODES, N_FEAT), np.float32)
    for c in range(NCORES):
        oc = np.asarray(res.results[c]["out"]).astype(np.float32)
        oc = oc.reshape(P, NB, N_FEAT).transpose(1, 0, 2)   # [NB, P, F]
        ng = node_grid[c]                                   # [NB, P]
        valid = ng >= 0
        out_full[ng[valid]] = oc[valid]
    return out_full, res


def kernel(**inputs):
    out, _ = _run(inputs["x"], inputs["edge_index"], trace=False)
    return out


# revision 36
# speedup vs baseline: 1.1105x; 1.0402x over previous
"""GNN message-passing (std aggregator) on 8 TRN2 NeuronCores.

Math per target node n: count, S1 = sum x[src], S2 = sum x[src]^2;
mean = S1/count; var = S2/count - mean^2; std = sqrt(max(var,0)),
zeroed where count <= 1.

Strategy (edge-major, identity-matmul segment-sum):
  Host sorts nodes by in-degree and assigns each node one SBUF lane:
  rank r -> (global block g = r//128, lane p = r%128); block g -> core
  g%8, per-core block index i = g//8. Per block-index capacity cap_i =
  max in-degree across the 8 interleaved global blocks (degree-sorted,
  so padding is a few %). Messages x[src] are shipped pre-gathered
  (host-side layout only) as bf16 slabs [128 lanes, cap_i*64] per
  block: column group j holds lane-node's j-th incoming message.

  Device per group of blocks: DMA slab; ACT squares it; PE accumulates
  S1 = sum_j msg_j and S2 = sum_j sq_j per lane with ONE wrapped-output
  matmul chain per block (identity stationary, out AP [128, c, 64] with
  stride-0 over c accumulates in PSUM); DVE finishing reads PSUM:
  t = S1*a, v = S2*a (a = mask/count shipped as bf16 plane),
  v = max(v - t*t, 0); ACT sqrt -> std; DMA out node-major.
  No per-edge descriptors, no collectives; every engine does large
  unit-stride work.
"""

import numpy as np
import ml_dtypes

N_NODES = 100000
N_FEAT = 64
P = 128
NCORES = 8
NBLK = 784                # global blocks (784*128 = 100352 >= 100000)
NB = NBLK // NCORES       # 98 per-core blocks
NRANK = NBLK * P
NGROUP = 16               # DMA/compute groups per core
MMC = 8                   # cap chunk per matmul (512 moving cols limit)

_CACHE = {}


def _build_program(caps, groups):
    import concourse.bass as bass
    import concourse.bacc as bacc
    import concourse.mybir as mybir
    import concourse.tile as tile

    F = N_FEAT
    BF16 = mybir.dt.bfloat16
    F32 = mybir.dt.float32
    AF = mybir.ActivationFunctionType
    AO = mybir.AluOpType

    offs = np.zeros(NB + 1, np.int64)
    np.cumsum(caps, out=offs[1:])
    tot = int(offs[-1])
    maxg = max(int(offs[b1] - offs[b0]) for b0, b1 in groups)
    maxgb = max(b1 - b0 for b0, b1 in groups)
    ACT_FRAC = 0.67          # fraction of squaring done on ScalarE vs DVE

    nc = bacc.Bacc()
    msgsd = nc.declare_dram_parameter("msgs", [P, tot * F], BF16, isOutput=False)
    arepd = nc.declare_dram_parameter("arep", [P, NB * 2 * F], BF16,
                                      isOutput=False)
    identd = nc.declare_dram_parameter("ident", [P, P], BF16, isOutput=False)
    outd = nc.declare_dram_parameter("out", [P, NB * F], BF16, isOutput=True)

    with tile.TileContext(nc) as tc:
        with (
            tc.tile_pool(name="const", bufs=1) as constp,
            tc.tile_pool(name="io", bufs=2) as iop,
            tc.tile_pool(name="sq", bufs=2) as sqp,
            tc.tile_pool(name="fin", bufs=2) as finp,
            tc.tile_pool(name="ov", bufs=2) as ovp,
            tc.tile_pool(name="ps", bufs=8, space="PSUM") as psump,
        ):
            ident = constp.tile([P, P], BF16)
            nc.sync.dma_start(out=ident[:], in_=identd[:, :])
            arep = constp.tile([P, NB * 2 * F], BF16)
            nc.sync.dma_start(out=arep[:], in_=arepd[:, :])

            for b0, b1 in groups:
                gcols = int(offs[b1] - offs[b0])
                gb = b1 - b0
                slab = iop.tile([P, maxg * F], BF16, tag="slab")
                nc.sync.dma_start(
                    out=slab[:, : gcols * F],
                    in_=msgsd[:, int(offs[b0]) * F : int(offs[b1]) * F],
                )
                sqs = sqp.tile([P, maxg * F], BF16, tag="sqs")
                cut = int(gcols * F * ACT_FRAC) // F * F
                nc.scalar.activation(
                    out=sqs[:, :cut], in_=slab[:, :cut], func=AF.Square)
                nc.vector.tensor_tensor(
                    out=sqs[:, cut : gcols * F], in0=slab[:, cut : gcols * F],
                    in1=slab[:, cut : gcols * F], op=AO.mult)

                pss = []
                boff = 0
                for b in range(b0, b1):
                    cap = int(caps[b])
                    ps = psump.tile([P, 2 * F], F32, tag="ps",
                                    name=f"ps_{b}")
                    pss.append(ps)
                    r3 = slab[:, boff * F : (boff + cap) * F].rearrange(
                        "p (c f) -> p c f", f=F)
                    s3 = sqs[:, boff * F : (boff + cap) * F].rearrange(
                        "p (c f) -> p c f", f=F)
                    nchunk = (cap + MMC - 1) // MMC
                    for half, m3 in ((0, r3), (1, s3)):
                        dst = ps[:, half * F : (half + 1) * F].rearrange(
                            "p (o f) -> p o f", o=1)
                        for k in range(nchunk):
                            sz = min(MMC, cap - k * MMC)
                            nc.tensor.matmul(
                                out=dst.to_broadcast([P, sz, F]),
                                lhsT=ident[:],
                                rhs=m3[:, k * MMC : k * MMC + sz, :],
                                start=(half == 0 and k == 0),
                                stop=(half == 1 and k == nchunk - 1),
                            )
                    boff += cap

                # tv[:, (bb, 0, f)] = S1*a (t), tv[:, (bb, 1, f)] = S2*a (v)
                tv = finp.tile([P, maxgb * 2 * F], F32, tag="tv")
                for bb, b in enumerate(range(b0, b1)):
                    nc.vector.tensor_tensor(
                        out=tv[:, bb * 2 * F : (bb + 1) * 2 * F],
                        in0=pss[bb][:, :], in1=arep[:, b * 2 * F : (b + 1) * 2 * F],
                        op=AO.mult)
                tv3 = tv[:, : gb * 2 * F].rearrange("p (b h f) -> p b h f",
                                                    h=2, f=F)
                th = tv3[:, :, 0, :]
                vh = tv3[:, :, 1, :]
                nc.vector.tensor_tensor(out=th, in0=th, in1=th, op=AO.mult)
                nc.vector.tensor_tensor(out=vh, in0=vh, in1=th, op=AO.subtract)
                nc.vector.tensor_scalar(out=vh, in0=vh, scalar1=0.0,
                                        scalar2=None, op0=AO.max)
                s = ovp.tile([P, maxgb * F], BF16, tag="s")
                nc.scalar.activation(out=s[:, : gb * F]
                                     .rearrange("p (b f) -> p b f", f=F),
                                     in_=vh, func=AF.Sqrt)
                nc.sync.dma_start(out=outd[:, b0 * F : b1 * F],
                                  in_=s[:, : gb * F])
    return nc


def _host_prep(x, edge_index):
    bf16 = ml_dtypes.bfloat16
    src = np.asarray(edge_index[0], dtype=np.int64)
    tgt = np.asarray(edge_index[1], dtype=np.int64)
    n_edges = src.shape[0]

    counts = np.bincount(tgt, minlength=N_NODES)
    order = np.argsort(-counts, kind="stable")          # rank -> node
    deg_r = np.zeros(NRANK, np.int64)
    deg_r[:N_NODES] = counts[order]
    rank = np.empty(N_NODES, np.int64)
    rank[order] = np.arange(N_NODES)

    caps = np.maximum(deg_r[np.arange(NB) * NCORES * P], 1)   # per block idx
    offs = np.zeros(NB + 1, np.int64)
    np.cumsum(caps, out=offs[1:])
    tot = int(offs[-1])

    # groups: contiguous blocks with ~equal total capacity
    target = tot / NGROUP
    groups = []
    b0 = 0
    acc = 0
    for b in range(NB):
        acc += caps[b]
        if acc >= target and b + 1 < NB or b == NB - 1:
            groups.append((b0, b + 1))
            b0 = b + 1
            acc = 0
    if b0 < NB:
        groups.append((b0, NB))

    # per-edge placement
    r_t = rank[tgt]
    eorder = np.argsort(r_t, kind="stable")
    rs = r_t[eorder]
    starts = np.zeros(NRANK, np.int64)
    np.cumsum(deg_r[:-1], out=starts[1:])
    j = np.arange(n_edges) - starts[rs]
    g = rs // P
    p = rs % P
    core = g % NCORES
    blk = g // NCORES
    col = offs[blk] + j
    srcs = src[eorder]

    xb = np.asarray(x, np.float32).astype(bf16)

    # per-node scale a = mask/count, node-major [P, NB*F]
    ranks_core = ((np.arange(NB)[:, None] * NCORES)[None, :, :]
                  + np.arange(NCORES)[:, None, None]) * P \
        + np.arange(P)[None, None, :]                   # [NCORES, NB, P]
    d_core = deg_r[ranks_core]                          # [NCORES, NB, P]
    a_core = np.where(d_core > 1, 1.0 / np.maximum(d_core, 1), 0.0)

    ident = np.eye(P, dtype=bf16)
    in_maps = []
    for c in range(NCORES):
        m = core == c
        buf = np.zeros((P, tot, N_FEAT), bf16)
        buf[p[m], col[m]] = xb[srcs[m]]
        arep = np.ascontiguousarray(
            np.broadcast_to(
                a_core[c].T[:, :, None, None], (P, NB, 2, N_FEAT)
            ).reshape(P, NB * 2 * N_FEAT).astype(bf16))
        in_maps.append({
            "msgs": buf.reshape(P, tot * N_FEAT),
            "arep": arep,
            "ident": ident,
        })

    # output mapping: node_grid[c, i, p] = node id (or -1 pad)
    order_pad = np.full(NRANK, -1, np.int64)
    order_pad[:N_NODES] = order
    node_grid = order_pad[ranks_core]                   # [NCORES, NB, P]
    return caps, groups, in_maps, node_grid


def _run(x, edge_index, trace=False):
    from concourse.bass_utils import run_bass_kernel_spmd

    caps, groups, in_maps, node_grid = _host_prep(x, edge_index)
    key = (tuple(int(c) for c in caps), tuple(groups))
    if key not in _CACHE:
        nc_ = _build_program(caps, groups)
        nc_.finalize()
        _CACHE[key] = nc_
    nc = _CACHE[key]
    res = run_bass_kernel_spmd(
        nc, in_maps, core_ids=list(range(NCORES)), trace=trace)

    out_full = np.empty((N_NODES, N_FEAT), np.float32)
    for c in range(NCORES):
        oc = np.asarray(res.results[c]["out"]).astype(np.float32)
        oc = oc.reshape(P, NB, N_FEAT).transpose(1, 0, 2)   # [NB, P, F]
        ng = node_grid[c]                                   # [NB, P]
        valid = ng >= 0
        out_full[ng[valid]] = oc[valid]
    return out_full, res


def kernel(**inputs):
    out, _ = _run(inputs["x"], inputs["edge_index"], trace=False)
    return out


# revision 46
# speedup vs baseline: 1.1725x; 1.0558x over previous
"""GNN message-passing (std aggregator) on 8 TRN2 NeuronCores.

Math per target node n: count, S1 = sum x[src], S2 = sum x[src]^2;
mean = S1/count; var = S2/count - mean^2; std = sqrt(max(var,0)),
zeroed where count <= 1.

Strategy (edge-major, fp8 DoubleRow identity-matmul segment-sum):
  Host sorts nodes by in-degree and assigns each node one SBUF lane:
  rank r -> (global block g = r//128, lane p = r%128); block g -> core
  g%8, per-core block index i = g//8. Per block-index capacity cap_i =
  max in-degree across the 8 interleaved global blocks (degree-sorted,
  so padding is a few %), rounded up to even. Messages x[src] are
  shipped pre-gathered (host-side layout only) as fp8-e4m3 slabs
  [128 lanes, cap_i*64] per block: column group j holds lane-node's
  j-th incoming message.

  Device per group of blocks: DMA slab; squares computed on a 3-way
  ACT/DVE/GPSIMD column split; PE accumulates S1 and S2 per lane with
  wrapped-output DoubleRow matmuls (pair-identity stationary [128,2,128]
  fp8 contracts 256 rows, out AP [128, c, 64] stride-0 accumulates in
  PSUM, 2 rows/cycle); DVE finishing reads PSUM: tv = [S1|S2] * a
  (a = mask/count shipped doubled as bf16 plane), v = max(v - t*t, 0);
  ACT sqrt -> std bf16; DMA out node-major. No per-edge descriptors,
  no collectives; every engine does large unit-stride work.
"""

import numpy as np
import ml_dtypes

N_NODES = 100000
N_FEAT = 64
P = 128
NCORES = 8
NBLK = 784                # global blocks (784*128 = 100352 >= 100000)
NB = NBLK // NCORES       # 98 per-core blocks
NRANK = NBLK * P
NGROUP = 16               # DMA/compute groups per core
MMC = 8                   # slots per S1 matmul (512 moving cycles)
MMCP = 8                  # pairs per S2 DoubleRow matmul (512 cycles)

ACT_FRAC = 0.67           # squaring share on ScalarE
GP_FRAC = 0.28            # squaring share on GPSIMD (rest on DVE)

_CACHE = {}


def _build_program(caps, groups):
    import concourse.bass as bass
    import concourse.bacc as bacc
    import concourse.mybir as mybir
    import concourse.tile as tile

    F = N_FEAT
    FP8S = mybir.dt.float8e4      # e4m3: squares (range to 448)
    BF16 = mybir.dt.bfloat16
    F32 = mybir.dt.float32
    AF = mybir.ActivationFunctionType
    AO = mybir.AluOpType
    PM = mybir.MatmulPerfMode

    offs = np.zeros(NB + 1, np.int64)
    np.cumsum(caps, out=offs[1:])
    tot = int(offs[-1])
    maxg = max(int(offs[b1] - offs[b0]) for b0, b1 in groups)
    maxgb = max(b1 - b0 for b0, b1 in groups)

    nc = bacc.Bacc()
    msgsd = nc.declare_dram_parameter("msgs", [P, tot * F], BF16, isOutput=False)
    ad = nc.declare_dram_parameter("a_sc", [P, NB], F32, isOutput=False)
    id3d = nc.declare_dram_parameter("ident3", [P, P], BF16, isOutput=False)
    identd = nc.declare_dram_parameter("identp", [P, 2 * P], FP8S,
                                       isOutput=False)
    outd = nc.declare_dram_parameter("out", [P, NB * F], BF16, isOutput=True)

    with tile.TileContext(nc) as tc:
        with (
            tc.tile_pool(name="const", bufs=1) as constp,
            tc.tile_pool(name="io", bufs=3) as iop,
            tc.tile_pool(name="sq", bufs=3) as sqp,
            tc.tile_pool(name="fin", bufs=2) as finp,
            tc.tile_pool(name="ov", bufs=2) as ovp,
            tc.tile_pool(name="ps", bufs=8, space="PSUM") as psump,
        ):
            ident3 = constp.tile([P, P], BF16)
            nc.sync.dma_start(out=ident3[:], in_=id3d[:, :])
            identp = constp.tile([P, 2 * P], FP8S)
            nc.sync.dma_start(out=identp[:], in_=identd[:, :])
            lhsp = identp[:].rearrange("p (q m) -> p q m", q=2)
            a_sc = constp.tile([P, NB], F32)
            nc.sync.dma_start(out=a_sc[:], in_=ad[:, :])

            for b0, b1 in groups:
                gcols = int(offs[b1] - offs[b0])
                gb = b1 - b0
                slab = iop.tile([P, maxg * F], BF16, tag="slab")
                nc.sync.dma_start(
                    out=slab[:, : gcols * F],
                    in_=msgsd[:, int(offs[b0]) * F : int(offs[b1]) * F],
                )
                sqs = sqp.tile([P, maxg * F], FP8S, tag="sqs")
                n = gcols * F
                c1 = int(n * ACT_FRAC) // F * F
                c2 = c1 + int(n * GP_FRAC) // F * F
                nc.scalar.activation(
                    out=sqs[:, :c1], in_=slab[:, :c1], func=AF.Square)
                nc.gpsimd.tensor_tensor(
                    out=sqs[:, c1:c2], in0=slab[:, c1:c2],
                    in1=slab[:, c1:c2], op=AO.mult)
                if c2 < n:
                    nc.vector.tensor_tensor(
                        out=sqs[:, c2:n], in0=slab[:, c2:n],
                        in1=slab[:, c2:n], op=AO.mult)

                # one block per PSUM bank; per-block tv-mult right after its
                # stop keeps banks recycling fast (DVE has no square work)
                tv = finp.tile([P, maxgb * 2 * F], F32, tag="tv")
                boff = 0
                for bb, b in enumerate(range(b0, b1)):
                    cap = int(caps[b])
                    npair = cap // 2
                    ps = psump.tile([P, 2 * F], F32, tag="ps",
                                    name=f"ps_{b}")
                    # S1: standard matmul over e3m4 message slots
                    r3 = slab[:, boff * F : (boff + cap) * F].rearrange(
                        "p (c f) -> p c f", f=F)
                    dst1 = ps[:, 0:F].rearrange("p (o f) -> p o f", o=1)
                    nch1 = (cap + MMC - 1) // MMC
                    for k in range(nch1):
                        sz = min(MMC, cap - k * MMC)
                        nc.tensor.matmul(
                            out=dst1.to_broadcast([P, sz, F]),
                            lhsT=ident3[:],
                            rhs=r3[:, k * MMC : k * MMC + sz, :],
                            start=(k == 0), stop=False,
                        )
                    # S2: DoubleRow over e4m3 squares (pair planes)
                    s4 = sqs[:, boff * F : (boff + cap) * F].rearrange(
                        "p (c q f) -> p q c f", q=2, f=F)
                    dst2 = ps[:, F : 2 * F].rearrange("p (o f) -> p o f", o=1)
                    nch2 = (npair + MMCP - 1) // MMCP
                    for k in range(nch2):
                        sz = min(MMCP, npair - k * MMCP)
                        nc.tensor.matmul(
                            out=dst2.to_broadcast([P, sz, F]),
                            lhsT=lhsp,
                            rhs=s4[:, :, k * MMCP : k * MMCP + sz, :],
                            start=False, stop=(k == nch2 - 1),
                            perf_mode=PM.DoubleRow,
                        )
                    boff += cap
                    nc.vector.tensor_scalar_mul(
                        out=tv[:, bb * 2 * F : (bb + 1) * 2 * F],
                        in0=ps[:, :], scalar1=a_sc[:, b : b + 1])
                tv3 = tv[:, : gb * 2 * F].rearrange("p (b h f) -> p b h f",
                                                    h=2, f=F)
                th = tv3[:, :, 0, :]
                vh = tv3[:, :, 1, :]
                nc.vector.tensor_tensor(out=th, in0=th, in1=th, op=AO.mult)
                nc.vector.tensor_tensor(out=vh, in0=vh, in1=th, op=AO.subtract)
                nc.vector.tensor_scalar(out=vh, in0=vh, scalar1=0.0,
                                        scalar2=None, op0=AO.max)
                s = ovp.tile([P, maxgb * F], BF16, tag="s")
                nc.scalar.activation(out=s[:, : gb * F]
                                     .rearrange("p (b f) -> p b f", f=F),
                                     in_=vh, func=AF.Sqrt)
                nc.sync.dma_start(out=outd[:, b0 * F : b1 * F],
                                  in_=s[:, : gb * F])
    return nc


def _host_prep(x, edge_index):
    bf16 = ml_dtypes.bfloat16
    fp8s = ml_dtypes.float8_e4m3fn
    src = np.asarray(edge_index[0], dtype=np.int64)
    tgt = np.asarray(edge_index[1], dtype=np.int64)
    n_edges = src.shape[0]

    counts = np.bincount(tgt, minlength=N_NODES)
    order = np.argsort(-counts, kind="stable")          # rank -> node
    deg_r = np.zeros(NRANK, np.int64)
    deg_r[:N_NODES] = counts[order]
    rank = np.empty(N_NODES, np.int64)
    rank[order] = np.arange(N_NODES)

    caps = np.maximum(deg_r[np.arange(NB) * NCORES * P], 2)   # per block idx
    caps = (caps + 1) // 2 * 2                                # even for pairs
    offs = np.zeros(NB + 1, np.int64)
    np.cumsum(caps, out=offs[1:])
    tot = int(offs[-1])

    # groups: contiguous blocks with ~equal total capacity
    target = tot / NGROUP
    groups = []
    b0 = 0
    acc = 0
    for b in range(NB):
        acc += caps[b]
        if acc >= target and b + 1 < NB or b == NB - 1:
            groups.append((b0, b + 1))
            b0 = b + 1
            acc = 0
    if b0 < NB:
        groups.append((b0, NB))

    # per-edge placement
    r_t = rank[tgt]
    eorder = np.argsort(r_t, kind="stable")
    rs = r_t[eorder]
    starts = np.zeros(NRANK, np.int64)
    np.cumsum(deg_r[:-1], out=starts[1:])
    j = np.arange(n_edges) - starts[rs]
    g = rs // P
    p = rs % P
    core = g % NCORES
    blk = g // NCORES
    col = offs[blk] + j
    srcs = src[eorder]

    xb = np.asarray(x, np.float32).astype(bf16)

    # per-node scale a = mask/count, node-major, doubled [P, NB*2F]
    ranks_core = ((np.arange(NB)[:, None] * NCORES)[None, :, :]
                  + np.arange(NCORES)[:, None, None]) * P \
        + np.arange(P)[None, None, :]                   # [NCORES, NB, P]
    d_core = deg_r[ranks_core]                          # [NCORES, NB, P]
    a_core = np.where(d_core > 1, 1.0 / np.maximum(d_core, 1), 0.0)

    ident3 = np.eye(P, dtype=bf16)
    identp = np.concatenate([np.eye(P), np.eye(P)], axis=1).astype(fp8s)
    in_maps = []
    for c in range(NCORES):
        m = core == c
        buf = np.zeros((P, tot, N_FEAT), bf16)
        buf[p[m], col[m]] = xb[srcs[m]]
        in_maps.append({
            "msgs": buf.reshape(P, tot * N_FEAT),
            "a_sc": np.ascontiguousarray(a_core[c].T.astype(np.float32)),
            "ident3": ident3,
            "identp": identp,
        })

    # output mapping: node_grid[c, i, p] = node id (or -1 pad)
    order_pad = np.full(NRANK, -1, np.int64)
    order_pad[:N_NODES] = order
    node_grid = order_pad[ranks_core]                   # [NCORES, NB, P]
    return caps, groups, in_maps, node_grid


def _run(x, edge_index, trace=False):
    from concourse.bass_utils import run_bass_kernel_spmd

    caps, groups, in_maps, node_grid = _host_prep(x, edge_index)
    key = (tuple(int(c) for c in caps), tuple(groups))
    if key not in _CACHE:
        nc_ = _build_program(caps, groups)
        nc_.finalize()
        _CACHE[key] = nc_
    nc = _CACHE[key]
    res = run_bass_kernel_spmd(
        nc, in_maps, core_ids=list(range(NCORES)), trace=trace)

    out_full = np.empty((N_NODES, N_FEAT), np.float32)
    for c in range(NCORES):
        oc = np.asarray(res.results[c]["out"]).astype(np.float32)
        oc = oc.reshape(P, NB, N_FEAT).transpose(1, 0, 2)   # [NB, P, F]
        ng = node_grid[c]                                   # [NB, P]
        valid = ng >= 0
        out_full[ng[valid]] = oc[valid]
    return out_full, res


def kernel(**inputs):
    out, _ = _run(inputs["x"], inputs["edge_index"], trace=False)
    return out


# revision 49
# speedup vs baseline: 1.3362x; 1.1397x over previous
"""GNN message-passing (std aggregator) on 8 TRN2 NeuronCores.

Math per target node n: count, S1 = sum x[src], S2 = sum x[src]^2;
mean = S1/count; var = S2/count - mean^2; std = sqrt(max(var,0)),
zeroed where count <= 1.

Strategy (edge-major, fp8 DoubleRow identity-matmul segment-sum):
  Host sorts nodes by in-degree and assigns each node one SBUF lane:
  rank r -> (global block g = r//128, lane p = r%128); block g -> core
  g%8, per-core block index i = g//8. Per block-index capacity cap_i =
  max in-degree across the 8 interleaved global blocks (degree-sorted,
  so padding is a few %), rounded up to even. Messages x[src] are
  shipped pre-gathered (host-side layout only) as fp8-e4m3 slabs
  [128 lanes, cap_i*64] per block: column group j holds lane-node's
  j-th incoming message.

  Device per group of blocks: DMA slab; squares computed on a 3-way
  ACT/DVE/GPSIMD column split; PE accumulates S1 and S2 per lane with
  wrapped-output DoubleRow matmuls (pair-identity stationary [128,2,128]
  fp8 contracts 256 rows, out AP [128, c, 64] stride-0 accumulates in
  PSUM, 2 rows/cycle); DVE finishing reads PSUM: tv = [S1|S2] * a
  (a = mask/count shipped doubled as bf16 plane), v = max(v - t*t, 0);
  ACT sqrt -> std bf16; DMA out node-major. No per-edge descriptors,
  no collectives; every engine does large unit-stride work.
"""

import numpy as np
import ml_dtypes

N_NODES = 100000
N_FEAT = 64
P = 128
NCORES = 8
NBLK = 784                # global blocks (784*128 = 100352 >= 100000)
NB = NBLK // NCORES       # 98 per-core blocks
NRANK = NBLK * P
NGROUP = 16               # DMA/compute groups per core
MMC = 8                   # slots per S1 matmul (512 moving cycles)
MMCP = 8                  # pairs per S2 DoubleRow matmul (512 cycles)

ACT_FRAC = 0.72           # squaring share on ScalarE
GP_FRAC = 0.28            # squaring share on GPSIMD (rest on DVE)

_CACHE = {}


def _build_program(caps, groups):
    import concourse.bass as bass
    import concourse.bacc as bacc
    import concourse.mybir as mybir
    import concourse.tile as tile

    F = N_FEAT
    FP8S = mybir.dt.float8e4      # e4m3: squares (range to 448)
    BF16 = mybir.dt.bfloat16
    F32 = mybir.dt.float32
    AF = mybir.ActivationFunctionType
    AO = mybir.AluOpType
    PM = mybir.MatmulPerfMode

    offs = np.zeros(NB + 1, np.int64)
    np.cumsum(caps, out=offs[1:])
    tot = int(offs[-1])
    maxg = max(int(offs[b1] - offs[b0]) for b0, b1 in groups)
    maxgb = max(b1 - b0 for b0, b1 in groups)

    nc = bacc.Bacc()
    msgsd = nc.declare_dram_parameter("msgs", [P, tot * F], BF16, isOutput=False)
    ad = nc.declare_dram_parameter("a_sc", [P, NB], F32, isOutput=False)
    id3d = nc.declare_dram_parameter("ident3", [P, P], BF16, isOutput=False)
    identd = nc.declare_dram_parameter("identp", [P, 2 * P], FP8S,
                                       isOutput=False)
    outd = nc.declare_dram_parameter("out", [P, NB * F], BF16, isOutput=True)

    with tile.TileContext(nc) as tc:
        with (
            tc.tile_pool(name="const", bufs=1) as constp,
            tc.tile_pool(name="io", bufs=3) as iop,
            tc.tile_pool(name="sq", bufs=3) as sqp,
            tc.tile_pool(name="fin", bufs=2) as finp,
            tc.tile_pool(name="ov", bufs=2) as ovp,
            tc.tile_pool(name="ps", bufs=8, space="PSUM") as psump,
        ):
            ident3 = constp.tile([P, P], BF16)
            nc.sync.dma_start(out=ident3[:], in_=id3d[:, :])
            identp = constp.tile([P, 2 * P], FP8S)
            nc.sync.dma_start(out=identp[:], in_=identd[:, :])
            lhsp = identp[:].rearrange("p (q m) -> p q m", q=2)
            a_sc = constp.tile([P, NB], F32)
            nc.sync.dma_start(out=a_sc[:], in_=ad[:, :])

            for b0, b1 in groups:
                gcols = int(offs[b1] - offs[b0])
                gb = b1 - b0
                slab = iop.tile([P, maxg * F], BF16, tag="slab")
                nc.sync.dma_start(
                    out=slab[:, : gcols * F],
                    in_=msgsd[:, int(offs[b0]) * F : int(offs[b1]) * F],
                )
                sqs = sqp.tile([P, maxg * F], FP8S, tag="sqs")
                n = gcols * F
                c1 = int(n * ACT_FRAC) // F * F
                c2 = c1 + int(n * GP_FRAC) // F * F
                nc.scalar.activation(
                    out=sqs[:, :c1], in_=slab[:, :c1], func=AF.Square)
                nc.gpsimd.tensor_tensor(
                    out=sqs[:, c1:c2], in0=slab[:, c1:c2],
                    in1=slab[:, c1:c2], op=AO.mult)
                if c2 < n:
                    nc.vector.tensor_tensor(
                        out=sqs[:, c2:n], in0=slab[:, c2:n],
                        in1=slab[:, c2:n], op=AO.mult)

                # one block per PSUM bank; per-block tv-mult right after its
                # stop keeps banks recycling fast. The mult writes strided
                # into two contiguous planes (t at 0, v at maxgb*F) so the
                # batched tail ops below run unit-stride.
                tv = finp.tile([P, 2 * maxgb * F], F32, tag="tv")
                tvv = tv[:].rearrange("p (h x) -> p h x", h=2)
                boff = 0
                for bb, b in enumerate(range(b0, b1)):
                    cap = int(caps[b])
                    npair = cap // 2
                    ps = psump.tile([P, 2 * F], F32, tag="ps",
                                    name=f"ps_{b}")
                    # S1: standard matmul over e3m4 message slots
                    r3 = slab[:, boff * F : (boff + cap) * F].rearrange(
                        "p (c f) -> p c f", f=F)
                    dst1 = ps[:, 0:F].rearrange("p (o f) -> p o f", o=1)
                    nch1 = (cap + MMC - 1) // MMC
                    for k in range(nch1):
                        sz = min(MMC, cap - k * MMC)
                        nc.tensor.matmul(
                            out=dst1.to_broadcast([P, sz, F]),
                            lhsT=ident3[:],
                            rhs=r3[:, k * MMC : k * MMC + sz, :],
                            start=(k == 0), stop=False,
                        )
                    # S2: DoubleRow over e4m3 squares (pair planes)
                    s4 = sqs[:, boff * F : (boff + cap) * F].rearrange(
                        "p (c q f) -> p q c f", q=2, f=F)
                    dst2 = ps[:, F : 2 * F].rearrange("p (o f) -> p o f", o=1)
                    nch2 = (npair + MMCP - 1) // MMCP
                    for k in range(nch2):
                        sz = min(MMCP, npair - k * MMCP)
                        nc.tensor.matmul(
                            out=dst2.to_broadcast([P, sz, F]),
                            lhsT=lhsp,
                            rhs=s4[:, :, k * MMCP : k * MMCP + sz, :],
                            start=False, stop=(k == nch2 - 1),
                            perf_mode=PM.DoubleRow,
                        )
                    boff += cap
                    nc.vector.tensor_scalar_mul(
                        out=tvv[:, :, bb * F : (bb + 1) * F],
                        in0=ps[:, :].rearrange("p (h f) -> p h f", h=2),
                        scalar1=a_sc[:, b : b + 1])
                th = tv[:, : gb * F]
                vh = tv[:, maxgb * F : maxgb * F + gb * F]
                nc.vector.tensor_tensor(out=th, in0=th, in1=th, op=AO.mult)
                nc.vector.tensor_tensor(out=vh, in0=vh, in1=th, op=AO.subtract)
                nc.vector.tensor_scalar(out=vh, in0=vh, scalar1=0.0,
                                        scalar2=None, op0=AO.max)
                s = ovp.tile([P, maxgb * F], BF16, tag="s")
                nc.scalar.activation(out=s[:, : gb * F], in_=vh, func=AF.Sqrt)
                nc.sync.dma_start(out=outd[:, b0 * F : b1 * F],
                                  in_=s[:, : gb * F])
    return nc


def _host_prep(x, edge_index):
    bf16 = ml_dtypes.bfloat16
    fp8s = ml_dtypes.float8_e4m3fn
    src = np.asarray(edge_index[0], dtype=np.int64)
    tgt = np.asarray(edge_index[1], dtype=np.int64)
    n_edges = src.shape[0]

    counts = np.bincount(tgt, minlength=N_NODES)
    order = np.argsort(-counts, kind="stable")          # rank -> node
    deg_r = np.zeros(NRANK, np.int64)
    deg_r[:N_NODES] = counts[order]
    rank = np.empty(N_NODES, np.int64)
    rank[order] = np.arange(N_NODES)

    caps = np.maximum(deg_r[np.arange(NB) * NCORES * P], 2)   # per block idx
    caps = (caps + 1) // 2 * 2                                # even for pairs
    offs = np.zeros(NB + 1, np.int64)
    np.cumsum(caps, out=offs[1:])
    tot = int(offs[-1])

    # groups: contiguous blocks with ~equal total capacity
    target = tot / NGROUP
    groups = []
    b0 = 0
    acc = 0
    for b in range(NB):
        acc += caps[b]
        if acc >= target and b + 1 < NB or b == NB - 1:
            groups.append((b0, b + 1))
            b0 = b + 1
            acc = 0
    if b0 < NB:
        groups.append((b0, NB))

    # per-edge placement
    r_t = rank[tgt]
    eorder = np.argsort(r_t, kind="stable")
    rs = r_t[eorder]
    starts = np.zeros(NRANK, np.int64)
    np.cumsum(deg_r[:-1], out=starts[1:])
    j = np.arange(n_edges) - starts[rs]
    g = rs // P
    p = rs % P
    core = g % NCORES
    blk = g // NCORES
    col = offs[blk] + j
    srcs = src[eorder]

    xb = np.asarray(x, np.float32).astype(bf16)

    # per-node scale a = mask/count, node-major, doubled [P, NB*2F]
    ranks_core = ((np.arange(NB)[:, None] * NCORES)[None, :, :]
                  + np.arange(NCORES)[:, None, None]) * P \
        + np.arange(P)[None, None, :]                   # [NCORES, NB, P]
    d_core = deg_r[ranks_core]                          # [NCORES, NB, P]
    a_core = np.where(d_core > 1, 1.0 / np.maximum(d_core, 1), 0.0)

    ident3 = np.eye(P, dtype=bf16)
    identp = np.concatenate([np.eye(P), np.eye(P)], axis=1).astype(fp8s)
    in_maps = []
    for c in range(NCORES):
        m = core == c
        buf = np.zeros((P, tot, N_FEAT), bf16)
        buf[p[m], col[m]] = xb[srcs[m]]
        in_maps.append({
            "msgs": buf.reshape(P, tot * N_FEAT),
            "a_sc": np.ascontiguousarray(a_core[c].T.astype(np.float32)),
            "ident3": ident3,
            "identp": identp,
        })

    # output mapping: node_grid[c, i, p] = node id (or -1 pad)
    order_pad = np.full(NRANK, -1, np.int64)
    order_pad[:N_NODES] = order
    node_grid = order_pad[ranks_core]                   # [NCORES, NB, P]
    return caps, groups, in_maps, node_grid


def _run(x, edge_index, trace=False):
    from concourse.bass_utils import run_bass_kernel_spmd

    caps, groups, in_maps, node_grid = _host_prep(x, edge_index)
    key = (tuple(int(c) for c in caps), tuple(groups))
    if key not in _CACHE:
        nc_ = _build_program(caps, groups)
        nc_.finalize()
        _CACHE[key] = nc_
    nc = _CACHE[key]
    res = run_bass_kernel_spmd(
        nc, in_maps, core_ids=list(range(NCORES)), trace=trace)

    out_full = np.empty((N_NODES, N_FEAT), np.float32)
    for c in range(NCORES):
        oc = np.asarray(res.results[c]["out"]).astype(np.float32)
        oc = oc.reshape(P, NB, N_FEAT).transpose(1, 0, 2)   # [NB, P, F]
        ng = node_grid[c]                                   # [NB, P]
        valid = ng >= 0
        out_full[ng[valid]] = oc[valid]
    return out_full, res


def kernel(**inputs):
    out, _ = _run(inputs["x"], inputs["edge_index"], trace=False)
    return out
